# revision 4
# baseline (speedup 1.0000x reference)
"""CLAHE (32,1,1024,1024) fp32 kernel for 8 TRN2 NeuronCores — v3.

Per core: 4 images = 256 tiles of 128x128 px, processed as 32 groups of
8 tiles (one tile row) in a deep software pipeline. Per tile: 256-bin
histogram (radix-16 one-hot matmuls on PE), clip at 128 + redistribute,
cumsum -> CDF table, per-pixel CDF lookup via GPSIMD ap_gather. v3:
  - per-group pipeline; table chain deferred 1 group, gathers 2 groups,
    so the DVE queue never head-blocks and the Pool engine stays ~91% busy
  - host-pretransposed x2 input: gather-layout idx via one DMA, no DRAM
    idx round trip
  - tabrep broadcast writes a PLAIN SBUF dest (factored-partition dests
    silently write a partition subset -- the old stale-table bug)
  - bit-exact vs the numpy reference on HW (absmax 0.0)

Self-contained: hardcodes shapes/sharding for image (32,1,1024,1024).
"""
import json
import os
import numpy as np

import concourse.bass as bass
import concourse.mybir as mybir
import concourse.tile as tile
from concourse import library_config
from concourse.bass_utils import run_bass_kernel_spmd
from concourse.library_overlay import lower_extended_insts

AO = mybir.AluOpType
DT = mybir.dt

N_CORES = 8
IMGS_PER_CORE = 4
H = W = 1024
TH = TW = 128
NBINS = 256
CSUB = 0.5 - 2.0 ** -16   # RNE(y - CSUB) == floor(y) for y on the 2^-15 grid
BLOCKS = 2                # 2 images per block -> 128 tiles/block
TPB = 128                 # tiles per block
COLS = TPB * TW           # 16384 px columns per block
PXCH = 1024               # px columns per bin-compute chunk (8 tiles = 1 group)
NPX = COLS // PXCH        # 16 chunks per block
OHCH = 512                # px columns per one-hot chunk
GCH = 4096                # idxs per ap_gather call


def _fix_drains(json_bytes: bytes) -> bytes:
    """walrus in this container allows only 1 sync-wait per instruction;
    split extra waits onto preceding same-engine Drain instructions."""
    m = json.loads(json_bytes)
    for f in m["functions"]:
        for blk in f["blocks"]:
            newl = []
            for ins in blk["instructions"]:
                si = ins.get("sync_info") or {}
                waits = si.get("on_wait") or []
                if len(waits) > 1 and "reset_range_start" not in ins:
                    for k, w in enumerate(waits[:-1]):
                        newl.append({
                            "debug": ins.get("debug", 0),
                            "engine": ins["engine"],
                            "ins": [], "outs": [],
                            "name": f'{ins["name"]}-w{k}',
                            "opcode": "Drain",
                            "sync_info": {"on_update": [], "on_wait": [w]},
                        })
                    ins["sync_info"]["on_wait"] = [waits[-1]]
                newl.append(ins)
            blk["instructions"] = newl
    return json.dumps(m).encode()


def build_nc():
    skip = set(os.environ.get("K_SKIP", "").split(","))
    reps = int(os.environ.get("K_REPS", "1"))
    nc = bass.Bass("TRN2", debug=False, num_devices=N_CORES)
    x = nc.dram_tensor("x", [IMGS_PER_CORE, H, W], DT.float32, kind="ExternalInput")
    # host-pretransposed gather-layout copy: [group, (tx pm), (ph q)]
    x2 = nc.dram_tensor("x2", [IMGS_PER_CORE * 8, 128, TH * TW // 16],
                        DT.float32, kind="ExternalInput")
    out = nc.dram_tensor("out", [BLOCKS * TPB, TH * TW], DT.float32,
                         kind="ExternalOutput")
    NSCR = 4   # rotate scratch tensors: tile tracks DRAM deps whole-tensor
    hist_scrs = [nc.dram_tensor(f"hist_scr{k}", [BLOCKS * TPB, NBINS],
                                DT.float32, kind="Internal")
                 for k in range(NSCR)]
    tab_scrs = [nc.dram_tensor(f"tab_scr{k}", [BLOCKS * TPB, NBINS],
                               DT.float32, kind="Internal")
                for k in range(NSCR)]

    # [p, img, ty, tx, q] view of the input (histogram layout)
    xv = x[:].rearrange("i (ty p) (tx q) -> p i ty tx q", p=TH, q=TW)
    # gather-layout view: [tx, pm, img, ty, ph, q]; tx outer so a plain
    # [128, .] SBUF dest factors its partition dim as (tx, pm) = tx*16+pm
    xw = x[:].rearrange("i (ty ph pm) (tx q) -> tx pm i ty ph q",
                        ph=8, pm=16, q=TW)

    with tile.TileContext(nc) as tc:
        with tc.tile_pool(name="px", bufs=2) as px_pool, \
             tc.tile_pool(name="oh", bufs=2) as oh_pool, \
             tc.tile_pool(name="ix", bufs=6) as ix_pool, \
             tc.tile_pool(name="sm", bufs=6) as sm_pool, \
             tc.tile_pool(name="tb", bufs=4) as tb_pool, \
             tc.tile_pool(name="gt", bufs=6) as gt_pool, \
             tc.tile_pool(name="go", bufs=4) as go_pool, \
             tc.tile_pool(name="ps", bufs=8, space="PSUM") as psp:
            with tc.tile_critical():
                nc.gpsimd.load_library(library_config.ap_gather)

            ctx = tc.For_i(0, reps) if reps > 1 else None
            if ctx is not None:
                ctx.__enter__()
            NGR = IMGS_PER_CORE * 8   # 32 groups of 8 tiles (one tile row)

            def emit_chain(g, stage8):
                # table chain for group g; emitted one group late so the
                # DVE queue head never stalls on the h-load DMA.
                r0 = g * 8
                hist_scr = hist_scrs[g % NSCR]
                tab_scr = tab_scrs[g % NSCR]
                nc.sync.dma_start(
                    hist_scr[r0:r0 + 8, :]
                    .rearrange("t (j1 j2) -> j1 t j2", j1=16), stage8[:])
                h = tb_pool.tile([8, NBINS], DT.float32, tag="h")
                nc.sync.dma_start(h[:], hist_scr[r0:r0 + 8, :])
                e = tb_pool.tile([8, NBINS], DT.float32, tag="e")
                nc.vector.tensor_scalar(e[:], h[:], 128.0, 0.0,
                                        AO.subtract, AO.max)
                E = tb_pool.tile([8, 1], DT.float32, tag="E")
                nc.vector.tensor_reduce(E[:], e[:],
                                        axis=mybir.AxisListType.X, op=AO.add)
                Es = tb_pool.tile([8, 1], DT.float32, tag="Es")
                nc.vector.tensor_scalar(Es[:], E[:], 1.0 / 256.0, None,
                                        AO.mult)
                hc = tb_pool.tile([8, NBINS], DT.float32, tag="hc")
                nc.vector.scalar_tensor_tensor(
                    hc[:], h[:], 128.0, Es[:].to_broadcast([8, NBINS]),
                    AO.min, AO.add)
                cdf = tb_pool.tile([8, NBINS], DT.float32, tag="cdf")
                nc.vector.tensor_tensor_scan(cdf[:], hc[:], hc[:], 0.0,
                                             AO.add, AO.bypass)
                tabt = tb_pool.tile([8, NBINS], DT.float32, tag="tabt")
                nc.vector.tensor_scalar(tabt[:], cdf[:], 1.0 / 16384.0,
                                        None, AO.mult)
                nc.sync.dma_start(tab_scr[r0:r0 + 8, :], tabt[:])
                tabrep = gt_pool.tile([128, NBINS], DT.float32, tag="tabrep")
                # dest partition dim must stay plain/unfactored; the (t, pm)
                # factorization rides on the broadcast source side.
                nc.sync.dma_start(
                    tabrep[:],
                    tab_scr[r0:r0 + 8, :]
                    .rearrange("t (o f) -> t o f", o=1)
                    .to_broadcast([8, 16, NBINS]))
                return tabrep

            def emit_gather(g, tabrep, idx16):
                # gathers for group g; deferred one further group so the
                # tabrep load leads its consumers by a full group (~23us)
                # even if a sync wait is dropped by the Drain splitter.
                r0 = g * 8
                for gc in range(TH * TW // GCH):
                    gout = go_pool.tile([128, GCH], DT.float32, tag="gout")
                    if "gather" in skip:
                        nc.vector.memset(gout[:, :1], 0.0)
                    else:
                        nc.gpsimd.ap_gather(
                            gout[:], tabrep[:],
                            idx16[:, gc * (GCH // 16):(gc + 1) * (GCH // 16)],
                            channels=128, num_elems=NBINS, d=1,
                            num_idxs=GCH)
                    nc.scalar.dma_start(
                        out[r0:r0 + 8, gc * GCH:(gc + 1) * GCH],
                        gout[:].rearrange("(t pm) f -> t pm f", pm=16)
                        [:, 0, :])

            pend_chain = None
            pend_gather = None

            def flush(new_chain):
                nonlocal pend_chain, pend_gather
                if pend_gather is not None:
                    emit_gather(*pend_gather)
                    pend_gather = None
                if pend_chain is not None:
                    cg, cstage, cidx = pend_chain
                    tabrep = emit_chain(cg, cstage)
                    pend_gather = (cg, tabrep, cidx)
                    pend_chain = None
                pend_chain = new_chain

            for g in range(NGR):
                t0 = g * 8                  # global tile index base
                img = t0 // 64
                ty = (t0 % 64) // 8
                r0 = g * 8                  # rows in hist_scr/tab_scr/out
                xc = px_pool.tile([128, PXCH], DT.float32, tag="xc")
                nc.sync.dma_start(
                    xc[:].rearrange("p (tx q) -> p tx q", q=TW),
                    xv[:, img, ty, :, :])
                b16 = px_pool.tile([128, PXCH], DT.int16, tag="b16")
                nc.vector.tensor_scalar(b16[:], xc[:], 256.0, CSUB,
                                        AO.mult, AO.subtract)
                hib = px_pool.tile([128, PXCH], DT.int16, tag="hib")
                nc.vector.tensor_scalar(hib[:], b16[:], 240, None,
                                        AO.bitwise_and)
                lob = px_pool.tile([128, PXCH], DT.int16, tag="lob")
                nc.vector.tensor_scalar(lob[:], b16[:], 15, None,
                                        AO.bitwise_and)
                # gather-layout copy of the same 8 tiles:
                # partition (tx pm), free (ph q)
                xc2 = px_pool.tile([128, PXCH], DT.float32, tag="xc2")
                nc.sync.dma_start(xc2[:], x2[g, :, :])
                # idx = floor(fp32(255*x)) exactly, in gather layout
                r16 = px_pool.tile([128, PXCH], DT.int16, tag="r16")
                nc.vector.tensor_scalar(r16[:], xc2[:], 255.0, CSUB,
                                        AO.mult, AO.subtract)
                d16 = px_pool.tile([128, PXCH], DT.int16, tag="d16")
                nc.vector.scalar_tensor_tensor(d16[:], xc2[:], 255.0,
                                               r16[:], AO.mult, AO.is_lt)
                idx16 = ix_pool.tile([128, PXCH], DT.int16, tag="idx16")
                nc.vector.tensor_tensor(idx16[:], r16[:], d16[:],
                                        AO.subtract)
                # one-hots + matmuls in OHCH sub-chunks
                stage8 = sm_pool.tile([16, 8, 16], DT.float32, tag="stage8")
                for sc in range(PXCH // OHCH):
                    off = sc * OHCH
                    OH = oh_pool.tile([128, 16, OHCH], DT.bfloat16, tag="OH")
                    OL = oh_pool.tile([128, 16, OHCH], DT.bfloat16, tag="OL")
                    jrange = [0] if "oh" in skip else list(range(16))
                    for j in jrange:
                        nc.vector.tensor_scalar(
                            OH[:, j, :], hib[:, off:off + OHCH],
                            16 * j, None, AO.is_equal)
                        nc.vector.tensor_scalar(
                            OL[:, j, :], lob[:, off:off + OHCH],
                            j, None, AO.is_equal)
                    for tl in range(OHCH // TW):
                        tloc = sc * (OHCH // TW) + tl
                        pt = psp.tile([16, 16], DT.float32, tag="acc")
                        if "mm" in skip:
                            nc.vector.memset(pt[:, :], 0.0)
                        else:
                            for c in range(TW):
                                col = tl * TW + c
                                nc.tensor.matmul(
                                    pt[:, :], OH[:, :, col], OL[:, :, col],
                                    start=(c == 0), stop=(c == TW - 1))
                        nc.vector.tensor_copy(stage8[:, tloc, :], pt[:, :])
                flush((g, stage8, idx16))
            flush(None)
            flush(None)
            if ctx is not None:
                ctx.__exit__(None, None, None)
    lower_extended_insts(nc)
    orig = nc.to_json_bytes
    nc.to_json_bytes = lambda: _fix_drains(orig())
    return nc


_NC_CACHE = None


def _get_nc():
    global _NC_CACHE
    if _NC_CACHE is None:
        _NC_CACHE = build_nc()
    return _NC_CACHE


def _unscramble(dev_out: np.ndarray) -> np.ndarray:
    """dev_out [256, 16384] per core, t = img*64+ty*8+tx,
    i = ph*2048 + q*16 + pm, pixel (row=ty*128+ph*16+pm, col=tx*128+q)."""
    a = dev_out.reshape(IMGS_PER_CORE, 8, 8, 8, 128, 16)  # img ty tx ph q pm
    a = a.transpose(0, 1, 3, 5, 2, 4)                     # img ty ph pm tx q
    return np.ascontiguousarray(a.reshape(IMGS_PER_CORE, H, W))


def _gather_layout(img4: np.ndarray) -> np.ndarray:
    """[4, 1024, 1024] -> [32, 128, 1024]: group (i ty), chan (tx pm),
    free (ph q)."""
    a = img4.reshape(IMGS_PER_CORE, 8, 8, 16, 8, TW)   # i ty ph pm tx q
    a = a.transpose(0, 1, 4, 3, 2, 5)                  # i ty tx pm ph q
    return np.ascontiguousarray(a.reshape(IMGS_PER_CORE * 8, 128, 8 * TW))


def kernel(image: np.ndarray) -> np.ndarray:
    image = np.asarray(image)
    B = image.shape[0]
    img = np.ascontiguousarray(image[:, 0].astype(np.float32, copy=False))
    nc = _get_nc()
    in_maps = [{"x": img[i * IMGS_PER_CORE:(i + 1) * IMGS_PER_CORE],
                "x2": _gather_layout(img[i * IMGS_PER_CORE:
                                         (i + 1) * IMGS_PER_CORE])}
               for i in range(N_CORES)]
    res = run_bass_kernel_spmd(nc, in_maps, core_ids=list(range(N_CORES)))
    outs = [_unscramble(res.results[i]["out"]) for i in range(N_CORES)]
    full = np.concatenate(outs, axis=0)
    return full.reshape(B, 1, H, W)


if __name__ == "__main__":
    rng = np.random.RandomState(0)
    img = rng.rand(32, 1, 1024, 1024).astype(np.float32)
    out = kernel(img)
    print(out.shape, out.dtype)
